# revision 10
# baseline (speedup 1.0000x reference)
"""Multi-head attention (B=16, N=1024, dim=768, H=12) on 8 TRN2 NeuronCores.

Sharding: pure data-parallel over batch (2 batches per core). Each core runs
the full attention block on its batch shard; no collectives.

Per-core dataflow (all layouts chosen so no on-device transposes are needed):
  - host pre-transposes x -> xT [768, 1024] per batch and qkv_w/proj_w -> w.T
  - QK projection computed in "T layout": qkT [j, n] (j = head-major rows)
  - V projection computed in natural layout v_nat [n, j] (x used as stationary
    operand), padded to 65 cols per head with a ones column so the attn@v
    matmul also produces the softmax denominator for free
  - scores computed transposed: scT[k, q] = kT.T @ qT, softmax-exp on ACT with
    the 1/sqrt(hd) scale fused (no max subtraction: |scores| <~ 8 for this
    data distribution, exp stays well inside fp32/bf16 range)
  - attn@v: out.T[hd+1, q] = v_nat.T @ expT, row 64 = denominator
  - batched reciprocal (custom DVE op), gpsimd partition-broadcast, in-place
    normalize
  - proj: y[n, dout] = outcatT.T @ projT; V-bias and proj bias folded into a
    single precomputed bias vector added on the way out of PSUM
Precision: f32r (s1e8m11) for the qkv-projection + scores path, bf16 for the
attention-weight/value/proj path (measured end-to-end ~3e-3 relative absmax
vs the fp32 reference, vs ~7e-3 for all-bf16).
"""

import sys

if "/opt/trn_rl_repo" not in sys.path:
    sys.path.insert(0, "/opt/trn_rl_repo")

import numpy as np
import ml_dtypes

N_CORES = 8
B, N, DIM = 16, 1024, 768
H, HD = 12, 64
J = 3 * DIM
SCALE = HD**-0.5
B_LOC = B // N_CORES  # 2 batches per core
NT = N // 128  # 8 n-tiles per batch
KC = DIM // 128  # 6 contraction chunks
JT_QK = 12  # q,k j-tiles (rows 0..1535 of qkv out)

# dtype config: "f32r" or "bf16" for the two halves of the pipeline
DT_QK_NAME = "f32r"  # x, wqkv, q/k activations (scores path)
DT_AV_NAME = "bf16"  # exp weights, v, outcat, wproj (attn-value path)

_BUILT = {}


def _round_f32r(a):
    """Round-to-nearest-even fp32 -> s1e8m11 (what the PE does for float32r)."""
    b = np.ascontiguousarray(a.astype(np.float32)).view(np.uint32)
    low = b & np.uint32(0xFFF)
    hi = b & np.uint32(0xFFFFF000)
    round_up = (low > 0x800) | ((low == 0x800) & (((hi >> 12) & 1) == 1))
    hi = hi + (round_up.astype(np.uint32) << 12)
    return hi.view(np.float32)


def _np_cast(a, name):
    if name == "f32r":
        return _round_f32r(a)
    if name == "bf16":
        return a.astype(ml_dtypes.bfloat16)
    return a.astype(np.float32)


def _build():
    import concourse.bacc as bacc
    import concourse.mybir as mybir
    import concourse.tile as tile

    F32 = mybir.dt.float32
    DT_QK = {"f32r": mybir.dt.float32r, "bf16": mybir.dt.bfloat16}[DT_QK_NAME]
    DT_AV = {"f32r": mybir.dt.float32r, "bf16": mybir.dt.bfloat16}[DT_AV_NAME]
    EXP = mybir.ActivationFunctionType.Exp
    MUL = mybir.AluOpType.mult
    ADD = mybir.AluOpType.add

    nc = bacc.Bacc("TRN2", target_bir_lowering=False, debug=False,
                   num_devices=N_CORES)

    xt_d = nc.dram_tensor("xt", [B_LOC, DIM, N], DT_QK, kind="ExternalInput")
    wqkv_d = nc.dram_tensor("wqkvT", [DIM, J], DT_QK, kind="ExternalInput")
    wproj_d = nc.dram_tensor("wprojT", [DIM, DIM], DT_AV, kind="ExternalInput")
    qkb_d = nc.dram_tensor("qkb", [128, JT_QK], F32, kind="ExternalInput")
    bproj_d = nc.dram_tensor("bproj", [1, DIM], F32, kind="ExternalInput")
    y_d = nc.dram_tensor("y", [B_LOC, N, DIM], F32, kind="ExternalOutput")

    with tile.TileContext(nc) as tc:
        with (
            tc.tile_pool(name="wpool", bufs=1) as wpool,
            tc.tile_pool(name="xtp", bufs=1) as xtp,
            tc.tile_pool(name="qkp", bufs=1) as qkp,
            tc.tile_pool(name="vp", bufs=1) as vp,
            tc.tile_pool(name="ocp", bufs=1) as ocp,
            tc.tile_pool(name="etp", bufs=2) as etp,
            tc.tile_pool(name="denp", bufs=1) as denp,
            tc.tile_pool(name="rbp", bufs=2) as rbp,
            tc.tile_pool(name="yp", bufs=2) as yp,
            tc.tile_pool(name="mmp", bufs=2, space="PSUM") as mmp,
            tc.tile_pool(name="scp", bufs=1, space="PSUM") as scp,
            tc.tile_pool(name="avp", bufs=2, space="PSUM") as avp,
        ):
            wqkv_sb = wpool.tile([128, KC, J], DT_QK)
            wproj_sb = wpool.tile([128, KC, DIM], DT_AV)
            qkb_sb = wpool.tile([128, JT_QK], F32)
            bias_bc = wpool.tile([128, DIM], F32)

            for kc in range(KC):
                nc.sync.dma_start(out=wqkv_sb[:, kc, :],
                                  in_=wqkv_d[kc * 128:(kc + 1) * 128, :])
            nc.sync.dma_start(out=qkb_sb[:], in_=qkb_d[:])
            nc.sync.dma_start(out=bias_bc[0:1, :], in_=bproj_d[:])
            for kc in range(KC):
                nc.sync.dma_start(out=wproj_sb[:, kc, :],
                                  in_=wproj_d[kc * 128:(kc + 1) * 128, :])
            nc.gpsimd.partition_broadcast(bias_bc[:], bias_bc[0:1, :])

            for b in range(B_LOC):
                xt_sb = xtp.tile([128, KC, N], DT_QK, tag="xt")
                for kc in range(KC):
                    nc.sync.dma_start(out=xt_sb[:, kc, :],
                                      in_=xt_d[b, kc * 128:(kc + 1) * 128, :])

                qkT = qkp.tile([128, JT_QK, N], DT_QK, tag="qkT")
                vnat = vp.tile([128, NT, H, HD + 1], DT_AV, tag="vnat")
                outcat = ocp.tile([128, KC, N], DT_AV, tag="outcat")
                # DVE writes must start at a partition multiple of 32, so
                # denominators are staged at bases {0,32,64,96} (head h ->
                # partition 32*(h//3), free block h%3) and DMA-repacked to
                # [H, N] for the batched reciprocal.
                den_st = denp.tile([97, 3 * N], F32, tag="denst")
                den_all = denp.tile([H, N], F32, tag="den")

                # ones column (col 64 of every head slot) for the denominator
                nc.vector.memset(vnat[:], 1.0)

                # ---- QK projection: qkT[j, n] = wqkvT.T @ xT (+ bias) ----
                for jt in range(JT_QK):
                    for nb in range(2):
                        ps = mmp.tile([128, 512], F32, tag="mm")
                        for kc in range(KC):
                            nc.tensor.matmul(
                                ps[:],
                                wqkv_sb[:, kc, jt * 128:(jt + 1) * 128],
                                xt_sb[:, kc, nb * 512:(nb + 1) * 512],
                                start=(kc == 0), stop=(kc == KC - 1),
                            )
                        nc.vector.tensor_scalar_add(
                            qkT[:, jt, nb * 512:(nb + 1) * 512], ps[:],
                            qkb_sb[:, jt:jt + 1])

                # ---- V projection (natural layout): v[n, j] = xT.T @ wqkvT ----
                for nt in range(NT):
                    for c0, cw in ((0, 512), (512, 256)):
                        ps = mmp.tile([128, 512], F32, tag="mm")
                        for kc in range(KC):
                            nc.tensor.matmul(
                                ps[:, 0:cw],
                                xt_sb[:, kc, nt * 128:(nt + 1) * 128],
                                wqkv_sb[:, kc, 2 * DIM + c0:2 * DIM + c0 + cw],
                                start=(kc == 0), stop=(kc == KC - 1),
                            )
                        nc.vector.tensor_copy(
                            vnat[:, nt, c0 // HD:(c0 + cw) // HD, 0:HD],
                            ps[:, 0:cw].rearrange("p (h d) -> p h d", d=HD),
                        )

                # ---- attention, one head-pair x q-half at a time ----
                for p in range(6):
                    hA, hB = 2 * p, 2 * p + 1
                    for s in range(2):
                        avA = avp.tile([HD + 1, 512], F32, tag="av")
                        avB = avp.tile([HD + 1, 512], F32, tag="av")
                        for kcp in range(4):
                            sc = scp.tile([128, 4, 512], F32, tag="sc")
                            for j in range(2):
                                kc = 2 * kcp + j
                                nc.tensor.matmul(
                                    sc[:, 2 * j, :],
                                    qkT[0:64, 6 + p, kc * 128:(kc + 1) * 128],
                                    qkT[0:64, p, s * 512:(s + 1) * 512],
                                    start=True, stop=True)
                                nc.tensor.matmul(
                                    sc[:, 2 * j + 1, :],
                                    qkT[64:128, 6 + p, kc * 128:(kc + 1) * 128],
                                    qkT[64:128, p, s * 512:(s + 1) * 512],
                                    start=True, stop=True)
                            et = etp.tile([128, 4, 512], DT_AV, tag="et")
                            nc.scalar.activation(et[:], sc[:], EXP, scale=SCALE)
                            for j in range(2):
                                kc = 2 * kcp + j
                                nc.tensor.matmul(
                                    avA[:], vnat[:, kc, hA, 0:HD + 1],
                                    et[:, 2 * j, :],
                                    start=(kc == 0), stop=(kc == 7))
                                nc.tensor.matmul(
                                    avB[:], vnat[:, kc, hB, 0:HD + 1],
                                    et[:, 2 * j + 1, :],
                                    start=(kc == 0), stop=(kc == 7))
                        for h, avt in ((hA, avA), (hB, avB)):
                            dp = 32 * (h // 3)
                            dc = (h % 3) * N + s * 512
                            nc.vector.tensor_copy(
                                den_st[dp:dp + 1, dc:dc + 512],
                                avt[HD:HD + 1, :])
                            nc.vector.tensor_copy(
                                outcat[(h % 2) * 64:(h % 2) * 64 + 64, h // 2,
                                       s * 512:(s + 1) * 512],
                                avt[0:HD, :])

                # ---- batched reciprocal + normalize ----
                for h in range(H):
                    dp = 32 * (h // 3)
                    dc = (h % 3) * N
                    nc.sync.dma_start(out=den_all[h:h + 1, :],
                                      in_=den_st[dp:dp + 1, dc:dc + N])
                recip = denp.tile([H, N], F32, tag="recip")
                nc.vector.reciprocal_approx_accurate(recip[:], den_all[:],
                                                     den_st[0:H, 0:N])
                for h in range(H):
                    # hop each recip row to partition 0 (gpsimd broadcast
                    # source), broadcast in place to all 128 partitions, and
                    # multiply through the matching 64-row half (tensor_tensor
                    # needs equal base partitions for its two SBUF inputs)
                    rb = rbp.tile([128, N], F32, tag="rb")
                    nc.sync.dma_start(out=rb[0:1, :], in_=recip[h:h + 1, :])
                    nc.gpsimd.partition_broadcast(rb[:], rb[0:1, :])
                    p0 = (h % 2) * 64
                    oc_ap = outcat[p0:p0 + 64, h // 2, :]
                    nc.vector.tensor_tensor(oc_ap, oc_ap, rb[p0:p0 + 64, :],
                                            MUL)

                # ---- output projection ----
                for nt in range(NT):
                    y_sb = yp.tile([128, DIM], F32, tag="y")
                    for c0, cw in ((0, 512), (512, 256)):
                        ps = mmp.tile([128, 512], F32, tag="mm")
                        for dc in range(KC):
                            nc.tensor.matmul(
                                ps[:, 0:cw],
                                outcat[:, dc, nt * 128:(nt + 1) * 128],
                                wproj_sb[:, dc, c0:c0 + cw],
                                start=(dc == 0), stop=(dc == KC - 1),
                            )
                        nc.vector.tensor_tensor(y_sb[:, c0:c0 + cw],
                                                ps[:, 0:cw],
                                                bias_bc[:, c0:c0 + cw], ADD)
                    nc.sync.dma_start(out=y_d[b, nt * 128:(nt + 1) * 128, :],
                                      in_=y_sb[:])

    nc.compile()
    return nc


def _get_nc():
    key = (DT_QK_NAME, DT_AV_NAME)
    if key not in _BUILT:
        _BUILT[key] = _build()
    return _BUILT[key]


def _prep_inputs(x, qkv_w, qkv_b, proj_w, proj_b):
    x = np.asarray(x, dtype=np.float32)
    qkv_w = np.asarray(qkv_w, dtype=np.float32)
    qkv_b = np.asarray(qkv_b, dtype=np.float32)
    proj_w = np.asarray(proj_w, dtype=np.float32)
    proj_b = np.asarray(proj_b, dtype=np.float32)

    wqkvT = _np_cast(np.ascontiguousarray(qkv_w.T), DT_QK_NAME)
    wprojT = _np_cast(np.ascontiguousarray(proj_w.T), DT_AV_NAME)
    qkb = np.ascontiguousarray(qkv_b[:1536].reshape(JT_QK, 128).T)
    bproj = (proj_b + qkv_b[2 * DIM:] @ proj_w.T).reshape(1, DIM)
    bproj = np.ascontiguousarray(bproj, dtype=np.float32)

    in_maps = []
    for c in range(N_CORES):
        xs = x[c * B_LOC:(c + 1) * B_LOC]  # [2, 1024, 768]
        xt = _np_cast(np.ascontiguousarray(xs.transpose(0, 2, 1)), DT_QK_NAME)
        in_maps.append({
            "xt": xt,
            "wqkvT": wqkvT,
            "wprojT": wprojT,
            "qkb": qkb,
            "bproj": bproj,
        })
    return in_maps


def run(x, qkv_w, qkv_b, proj_w, proj_b, **spmd_kwargs):
    """Execute on 8 cores; returns (output, BassKernelResults)."""
    from concourse.bass_utils import run_bass_kernel_spmd

    nc = _get_nc()
    in_maps = _prep_inputs(x, qkv_w, qkv_b, proj_w, proj_b)
    res = run_bass_kernel_spmd(nc, in_maps, core_ids=list(range(N_CORES)),
                               **spmd_kwargs)
    y = np.concatenate([res.results[c]["y"] for c in range(N_CORES)], axis=0)
    return y.astype(np.float32), res


def kernel(x, qkv_w, qkv_b, proj_w, proj_b):
    y, _ = run(x, qkv_w, qkv_b, proj_w, proj_b)
    return y


# revision 13
# speedup vs baseline: 1.3616x; 1.3616x over previous
"""Multi-head attention (B=16, N=1024, dim=768, H=12) on 8 TRN2 NeuronCores.

Sharding: pure data-parallel over batch (2 batches per core). Each core runs
the full attention block on its batch shard; no collectives.

Per-core dataflow (all layouts chosen so no on-device transposes are needed):
  - host pre-transposes x -> xT [768, 1024] per batch and qkv_w/proj_w -> w.T
  - QK projection computed in "T layout": qkT [j, n] (j = head-major rows)
  - V projection computed in natural layout v_nat [n, j] (x used as stationary
    operand), padded to 65 cols per head with a ones column so the attn@v
    matmul also produces the softmax denominator for free
  - scores computed transposed: scT[k, q] = kT.T @ qT, softmax-exp on ACT with
    the 1/sqrt(hd) scale fused (no max subtraction: |scores| <~ 8 for this
    data distribution, exp stays well inside fp32/bf16 range)
  - attn@v: out.T[hd+1, q] = v_nat.T @ expT, row 64 = denominator
  - batched reciprocal (custom DVE op), gpsimd partition-broadcast, in-place
    normalize
  - proj: y[n, dout] = outcatT.T @ projT; V-bias and proj bias folded into a
    single precomputed bias vector added on the way out of PSUM
Precision: f32r (s1e8m11) for the qkv-projection + scores path, bf16 for the
attention-weight/value/proj path (measured end-to-end ~3e-3 relative absmax
vs the fp32 reference, vs ~7e-3 for all-bf16).
"""

import sys

if "/opt/trn_rl_repo" not in sys.path:
    sys.path.insert(0, "/opt/trn_rl_repo")

import numpy as np
import ml_dtypes

N_CORES = 8
B, N, DIM = 16, 1024, 768
H, HD = 12, 64
J = 3 * DIM
SCALE = HD**-0.5
B_LOC = B // N_CORES  # 2 batches per core
NT = N // 128  # 8 n-tiles per batch
KC = DIM // 128  # 6 contraction chunks
JT_QK = 12  # q,k j-tiles (rows 0..1535 of qkv out)

# dtype config: "f32r" or "bf16" for the two halves of the pipeline
DT_QK_NAME = "f32r"  # x, wqkv, q/k activations (scores path)
DT_AV_NAME = "bf16"  # exp weights, v, outcat, wproj (attn-value path)

_BUILT = {}


def _round_f32r(a):
    """Round-to-nearest-even fp32 -> s1e8m11 (what the PE does for float32r)."""
    b = np.ascontiguousarray(a.astype(np.float32)).view(np.uint32)
    low = b & np.uint32(0xFFF)
    hi = b & np.uint32(0xFFFFF000)
    round_up = (low > 0x800) | ((low == 0x800) & (((hi >> 12) & 1) == 1))
    hi = hi + (round_up.astype(np.uint32) << 12)
    return hi.view(np.float32)


def _np_cast(a, name):
    if name == "f32r":
        return _round_f32r(a)
    if name == "bf16":
        return a.astype(ml_dtypes.bfloat16)
    return a.astype(np.float32)


def _build():
    import concourse.bacc as bacc
    import concourse.mybir as mybir
    import concourse.tile as tile

    F32 = mybir.dt.float32
    DT_QK = {"f32r": mybir.dt.float32r, "bf16": mybir.dt.bfloat16}[DT_QK_NAME]
    DT_AV = {"f32r": mybir.dt.float32r, "bf16": mybir.dt.bfloat16}[DT_AV_NAME]
    EXP = mybir.ActivationFunctionType.Exp
    MUL = mybir.AluOpType.mult
    ADD = mybir.AluOpType.add

    nc = bacc.Bacc("TRN2", target_bir_lowering=False, debug=False,
                   num_devices=N_CORES)

    xt_d = nc.dram_tensor("xt", [B_LOC, DIM, N], DT_QK, kind="ExternalInput")
    wqkv_d = nc.dram_tensor("wqkvT", [DIM, J], DT_QK, kind="ExternalInput")
    wproj_d = nc.dram_tensor("wprojT", [DIM, DIM], DT_AV, kind="ExternalInput")
    qkb_d = nc.dram_tensor("qkb", [128, JT_QK], F32, kind="ExternalInput")
    bproj_d = nc.dram_tensor("bproj", [1, DIM], F32, kind="ExternalInput")
    y_d = nc.dram_tensor("y", [B_LOC, N, DIM], F32, kind="ExternalOutput")

    with tile.TileContext(nc) as tc:
        with (
            tc.tile_pool(name="wpool", bufs=1) as wpool,
            tc.tile_pool(name="xtp", bufs=1) as xtp,
            tc.tile_pool(name="qkp", bufs=1) as qkp,
            tc.tile_pool(name="vp", bufs=1) as vp,
            tc.tile_pool(name="ocp", bufs=1) as ocp,
            tc.tile_pool(name="etp", bufs=2) as etp,
            tc.tile_pool(name="denp", bufs=1) as denp,
            tc.tile_pool(name="rbp", bufs=1) as rbp,
            tc.tile_pool(name="yp", bufs=2) as yp,
            tc.tile_pool(name="mmp", bufs=2, space="PSUM") as mmp,
            tc.tile_pool(name="scp", bufs=2, space="PSUM") as scp,
            tc.tile_pool(name="avp", bufs=2, space="PSUM") as avp,
        ):
            wqkv_sb = wpool.tile([128, KC, J], DT_QK)
            wproj_sb = wpool.tile([128, KC, DIM], DT_AV)
            qkb_sb = wpool.tile([128, JT_QK], F32)
            bias_bc = wpool.tile([128, DIM], F32)

            nc.sync.dma_start(out=qkb_sb[:], in_=qkb_d[:])
            nc.sync.dma_start(out=bias_bc[0:1, :], in_=bproj_d[:])
            nc.gpsimd.partition_broadcast(bias_bc[:], bias_bc[0:1, :])
            for kc in range(KC):
                nc.sync.dma_start(out=wqkv_sb[:, kc, :],
                                  in_=wqkv_d[kc * 128:(kc + 1) * 128, :])
            for kc in range(KC):
                nc.sync.dma_start(out=wproj_sb[:, kc, :],
                                  in_=wproj_d[kc * 128:(kc + 1) * 128, :])

            st = {}  # per-batch tiles

            def load(b):
                xt_sb = xtp.tile([128, KC, N], DT_QK, tag="xt")
                for kc in range(KC):
                    nc.sync.dma_start(out=xt_sb[:, kc, :],
                                      in_=xt_d[b, kc * 128:(kc + 1) * 128, :])
                st[b] = {"xt": xt_sb}

            def qkv(b):
                s_ = st[b]
                xt_sb = s_["xt"]
                qkT = qkp.tile([128, JT_QK, N], DT_QK, tag="qkT")
                vnat = vp.tile([128, NT, H, HD + 1], DT_AV, tag="vnat")
                # ones column (col 64 of every head slot) for the denominator
                nc.vector.memset(vnat[:], 1.0)
                for jt in range(JT_QK):
                    for nb in range(2):
                        ps = mmp.tile([128, 512], F32, tag="mm")
                        for kc in range(KC):
                            nc.tensor.matmul(
                                ps[:],
                                wqkv_sb[:, kc, jt * 128:(jt + 1) * 128],
                                xt_sb[:, kc, nb * 512:(nb + 1) * 512],
                                start=(kc == 0), stop=(kc == KC - 1),
                            )
                        nc.vector.tensor_scalar_add(
                            qkT[:, jt, nb * 512:(nb + 1) * 512], ps[:],
                            qkb_sb[:, jt:jt + 1])
                for nt in range(NT):
                    for c0, cw in ((0, 512), (512, 256)):
                        ps = mmp.tile([128, 512], F32, tag="mm")
                        for kc in range(KC):
                            nc.tensor.matmul(
                                ps[:, 0:cw],
                                xt_sb[:, kc, nt * 128:(nt + 1) * 128],
                                wqkv_sb[:, kc, 2 * DIM + c0:2 * DIM + c0 + cw],
                                start=(kc == 0), stop=(kc == KC - 1),
                            )
                        nc.vector.tensor_copy(
                            vnat[:, nt, c0 // HD:(c0 + cw) // HD, 0:HD],
                            ps[:, 0:cw].rearrange("p (h d) -> p h d", d=HD),
                        )
                s_["qkT"] = qkT
                s_["vnat"] = vnat

            # den staging: DVE writes must start at a partition multiple of
            # 32, so head h's denominator goes to partition 32*(h//3), free
            # block h%3; then per-head DMAs repack into den_lo/den_hi rows
            # 0..5 (custom-DVE reciprocal only works at partition base 0).
            def norm_half(b, hlo):
                s_ = st[b]
                den_all = s_["den_lo" if hlo == 0 else "den_hi"]
                recip = s_["recip_lo" if hlo == 0 else "recip_hi"]
                outcat = s_["outcat"]
                nc.vector.reciprocal_approx_accurate(
                    recip[:], den_all[:], s_["den_st"][0:6, 0:N])
                for h in range(hlo, hlo + 6):
                    rb = rbp.tile([128, N], F32, tag="rb")
                    rr = h % 6
                    nc.sync.dma_start(out=rb[0:1, :], in_=recip[rr:rr + 1, :])
                    nc.gpsimd.partition_broadcast(rb[:], rb[0:1, :])
                    p0 = (h % 2) * 64
                    oc_ap = outcat[p0:p0 + 64, h // 2, :]
                    nc.vector.tensor_tensor(oc_ap, oc_ap, rb[p0:p0 + 64, :],
                                            MUL)

            def attn(b):
                s_ = st[b]
                qkT, vnat = s_["qkT"], s_["vnat"]
                outcat = ocp.tile([128, KC, N], DT_AV, tag="outcat")
                den_st = denp.tile([97, 3 * N], F32, tag="denst")
                den_lo = denp.tile([6, N], F32, tag="denlo")
                den_hi = denp.tile([6, N], F32, tag="denhi")
                recip_lo = denp.tile([6, N], F32, tag="reciplo")
                recip_hi = denp.tile([6, N], F32, tag="reciphi")
                s_.update(outcat=outcat, den_st=den_st, den_lo=den_lo,
                          den_hi=den_hi, recip_lo=recip_lo, recip_hi=recip_hi)
                for h in range(H):
                    p0 = (h % 2) * 64
                    pair = h // 2
                    for s in range(2):
                        avh = avp.tile([HD + 1, 512], F32, tag="av")
                        for g in range(4):
                            sc = scp.tile([128, 2, 512], F32, tag="sc")
                            for i in range(2):
                                kc = 2 * g + i
                                nc.tensor.matmul(
                                    sc[:, i, :],
                                    qkT[p0:p0 + 64, 6 + pair,
                                        kc * 128:(kc + 1) * 128],
                                    qkT[p0:p0 + 64, pair,
                                        s * 512:(s + 1) * 512],
                                    start=True, stop=True)
                            et = etp.tile([128, 2, 512], DT_AV, tag="et")
                            nc.scalar.activation(et[:], sc[:], EXP, scale=SCALE)
                            for i in range(2):
                                kc = 2 * g + i
                                nc.tensor.matmul(
                                    avh[:], vnat[:, kc, h, 0:HD + 1],
                                    et[:, i, :],
                                    start=(kc == 0), stop=(kc == 7))
                        dp = 32 * (h // 3)
                        dc = (h % 3) * N + s * 512
                        nc.vector.tensor_copy(den_st[dp:dp + 1, dc:dc + 512],
                                              avh[HD:HD + 1, :])
                        nc.vector.tensor_copy(
                            outcat[p0:p0 + 64, pair, s * 512:(s + 1) * 512],
                            avh[0:HD, :])
                    dtile = den_lo if h < 6 else den_hi
                    rr = h % 6
                    dp = 32 * (h // 3)
                    dc = (h % 3) * N
                    nc.sync.dma_start(out=dtile[rr:rr + 1, :],
                                      in_=den_st[dp:dp + 1, dc:dc + N])
                    if h == 5:
                        norm_half(b, 0)

            def proj(b):
                s_ = st[b]
                outcat = s_["outcat"]
                for nt in range(NT):
                    y_sb = yp.tile([128, DIM], F32, tag="y")
                    for c0, cw in ((0, 512), (512, 256)):
                        ps = mmp.tile([128, 512], F32, tag="mm")
                        for dc in range(KC):
                            nc.tensor.matmul(
                                ps[:, 0:cw],
                                outcat[:, dc, nt * 128:(nt + 1) * 128],
                                wproj_sb[:, dc, c0:c0 + cw],
                                start=(dc == 0), stop=(dc == KC - 1),
                            )
                        nc.vector.tensor_tensor(y_sb[:, c0:c0 + cw],
                                                ps[:, 0:cw],
                                                bias_bc[:, c0:c0 + cw], ADD)
                    nc.sync.dma_start(out=y_d[b, nt * 128:(nt + 1) * 128, :],
                                      in_=y_sb[:])

            # phase order chosen so batch 1's qkv fills batch 0's
            # normalize/proj pipeline gaps
            load(0)
            qkv(0)
            attn(0)
            load(1)
            qkv(1)
            norm_half(0, 6)
            proj(0)
            attn(1)
            norm_half(1, 6)
            proj(1)

    nc.compile()
    return nc


def _get_nc():
    key = (DT_QK_NAME, DT_AV_NAME)
    if key not in _BUILT:
        _BUILT[key] = _build()
    return _BUILT[key]


def _prep_inputs(x, qkv_w, qkv_b, proj_w, proj_b):
    x = np.asarray(x, dtype=np.float32)
    qkv_w = np.asarray(qkv_w, dtype=np.float32)
    qkv_b = np.asarray(qkv_b, dtype=np.float32)
    proj_w = np.asarray(proj_w, dtype=np.float32)
    proj_b = np.asarray(proj_b, dtype=np.float32)

    wqkvT = _np_cast(np.ascontiguousarray(qkv_w.T), DT_QK_NAME)
    wprojT = _np_cast(np.ascontiguousarray(proj_w.T), DT_AV_NAME)
    qkb = np.ascontiguousarray(qkv_b[:1536].reshape(JT_QK, 128).T)
    bproj = (proj_b + qkv_b[2 * DIM:] @ proj_w.T).reshape(1, DIM)
    bproj = np.ascontiguousarray(bproj, dtype=np.float32)

    in_maps = []
    for c in range(N_CORES):
        xs = x[c * B_LOC:(c + 1) * B_LOC]  # [2, 1024, 768]
        xt = _np_cast(np.ascontiguousarray(xs.transpose(0, 2, 1)), DT_QK_NAME)
        in_maps.append({
            "xt": xt,
            "wqkvT": wqkvT,
            "wprojT": wprojT,
            "qkb": qkb,
            "bproj": bproj,
        })
    return in_maps


def run(x, qkv_w, qkv_b, proj_w, proj_b, **spmd_kwargs):
    """Execute on 8 cores; returns (output, BassKernelResults)."""
    from concourse.bass_utils import run_bass_kernel_spmd

    nc = _get_nc()
    in_maps = _prep_inputs(x, qkv_w, qkv_b, proj_w, proj_b)
    res = run_bass_kernel_spmd(nc, in_maps, core_ids=list(range(N_CORES)),
                               **spmd_kwargs)
    y = np.concatenate([res.results[c]["y"] for c in range(N_CORES)], axis=0)
    return y.astype(np.float32), res


def kernel(x, qkv_w, qkv_b, proj_w, proj_b):
    y, _ = run(x, qkv_w, qkv_b, proj_w, proj_b)
    return y


# revision 14
# speedup vs baseline: 1.5051x; 1.1054x over previous
"""Multi-head attention (B=16, N=1024, dim=768, H=12) on 8 TRN2 NeuronCores.

Sharding: pure data-parallel over batch (2 batches per core). Each core runs
the full attention block on its batch shard; no collectives.

Per-core dataflow (all layouts chosen so no on-device transposes are needed):
  - host pre-transposes x -> xT [768, 1024] per batch and qkv_w/proj_w -> w.T
  - QK projection computed in "T layout": qkT [j, n] (j = head-major rows)
  - V projection computed in natural layout v_nat [n, j] (x used as stationary
    operand), padded to 65 cols per head with a ones column so the attn@v
    matmul also produces the softmax denominator for free
  - scores computed transposed: scT[k, q] = kT.T @ qT, softmax-exp on ACT with
    the 1/sqrt(hd) scale fused (no max subtraction: |scores| <~ 8 for this
    data distribution, exp stays well inside fp32/bf16 range)
  - attn@v: out.T[hd+1, q] = v_nat.T @ expT, row 64 = denominator
  - batched reciprocal (custom DVE op), gpsimd partition-broadcast, in-place
    normalize
  - proj: y[n, dout] = outcatT.T @ projT; V-bias and proj bias folded into a
    single precomputed bias vector added on the way out of PSUM
Precision: f32r (s1e8m11) for the qkv-projection + scores path, bf16 for the
attention-weight/value/proj path (measured end-to-end ~3e-3 relative absmax
vs the fp32 reference, vs ~7e-3 for all-bf16).
"""

import sys

if "/opt/trn_rl_repo" not in sys.path:
    sys.path.insert(0, "/opt/trn_rl_repo")

import numpy as np
import ml_dtypes

N_CORES = 8
B, N, DIM = 16, 1024, 768
H, HD = 12, 64
J = 3 * DIM
SCALE = HD**-0.5
B_LOC = B // N_CORES  # 2 batches per core
NT = N // 128  # 8 n-tiles per batch
KC = DIM // 128  # 6 contraction chunks
JT_QK = 12  # q,k j-tiles (rows 0..1535 of qkv out)

# dtype config: "f32r" or "bf16" for the two halves of the pipeline
DT_QK_NAME = "f32r"  # x, wqkv, q/k activations (scores path)
DT_AV_NAME = "bf16"  # exp weights, v, outcat, wproj (attn-value path)

_BUILT = {}


def _round_f32r(a):
    """Round-to-nearest-even fp32 -> s1e8m11 (what the PE does for float32r)."""
    b = np.ascontiguousarray(a.astype(np.float32)).view(np.uint32)
    low = b & np.uint32(0xFFF)
    hi = b & np.uint32(0xFFFFF000)
    round_up = (low > 0x800) | ((low == 0x800) & (((hi >> 12) & 1) == 1))
    hi = hi + (round_up.astype(np.uint32) << 12)
    return hi.view(np.float32)


def _np_cast(a, name):
    if name == "f32r":
        return _round_f32r(a)
    if name == "bf16":
        return a.astype(ml_dtypes.bfloat16)
    return a.astype(np.float32)


def _build():
    import concourse.bacc as bacc
    import concourse.mybir as mybir
    import concourse.tile as tile

    F32 = mybir.dt.float32
    DT_QK = {"f32r": mybir.dt.float32r, "bf16": mybir.dt.bfloat16}[DT_QK_NAME]
    DT_AV = {"f32r": mybir.dt.float32r, "bf16": mybir.dt.bfloat16}[DT_AV_NAME]
    EXP = mybir.ActivationFunctionType.Exp
    MUL = mybir.AluOpType.mult
    ADD = mybir.AluOpType.add

    nc = bacc.Bacc("TRN2", target_bir_lowering=False, debug=False,
                   num_devices=N_CORES)

    xt_d = nc.dram_tensor("xt", [B_LOC, DIM, N], DT_QK, kind="ExternalInput")
    wqkv_d = nc.dram_tensor("wqkvT", [DIM, J], DT_QK, kind="ExternalInput")
    wproj_d = nc.dram_tensor("wprojT", [DIM, DIM], DT_AV, kind="ExternalInput")
    qkb_d = nc.dram_tensor("qkb", [128, JT_QK], F32, kind="ExternalInput")
    bproj_d = nc.dram_tensor("bproj", [1, DIM], F32, kind="ExternalInput")
    y_d = nc.dram_tensor("y", [B_LOC, N, DIM], F32, kind="ExternalOutput")

    with tile.TileContext(nc) as tc:
        with (
            tc.tile_pool(name="wpool", bufs=1) as wpool,
            tc.tile_pool(name="xtp", bufs=1) as xtp,
            tc.tile_pool(name="qkp", bufs=1) as qkp,
            tc.tile_pool(name="vp", bufs=1) as vp,
            tc.tile_pool(name="ocp", bufs=1) as ocp,
            tc.tile_pool(name="etp", bufs=2) as etp,
            tc.tile_pool(name="denp", bufs=1) as denp,
            tc.tile_pool(name="rbp", bufs=2) as rbp,
            tc.tile_pool(name="yp", bufs=1) as yp,
            tc.tile_pool(name="mmp", bufs=2, space="PSUM") as mmp,
            tc.tile_pool(name="scp", bufs=2, space="PSUM") as scp,
            tc.tile_pool(name="avp", bufs=2, space="PSUM") as avp,
        ):
            wqkv_sb = wpool.tile([128, KC, J], DT_QK)
            wproj_sb = wpool.tile([128, KC, DIM], DT_AV)
            qkb_sb = wpool.tile([128, JT_QK], F32)
            bias_bc = wpool.tile([128, DIM], F32)

            nc.sync.dma_start(out=qkb_sb[:], in_=qkb_d[:])
            nc.sync.dma_start(out=bias_bc[0:1, :], in_=bproj_d[:])
            nc.gpsimd.partition_broadcast(bias_bc[:], bias_bc[0:1, :])

            st = {}  # per-batch tiles

            def load(b, with_weights=False):
                xt_sb = xtp.tile([128, KC, N], DT_QK, tag="xt")
                for kc in range(KC):
                    if with_weights:
                        nc.sync.dma_start(out=wqkv_sb[:, kc, :],
                                          in_=wqkv_d[kc * 128:(kc + 1) * 128, :])
                    nc.sync.dma_start(out=xt_sb[:, kc, :],
                                      in_=xt_d[b, kc * 128:(kc + 1) * 128, :])
                if with_weights:
                    for kc in range(KC):
                        nc.sync.dma_start(out=wproj_sb[:, kc, :],
                                          in_=wproj_d[kc * 128:(kc + 1) * 128, :])
                st[b] = {"xt": xt_sb}

            def qkv(b):
                s_ = st[b]
                xt_sb = s_["xt"]
                qkT = qkp.tile([128, JT_QK, N], DT_QK, tag="qkT")
                vnat = vp.tile([128, NT, H, HD + 1], DT_AV, tag="vnat")
                # ones column (col 64 of every head slot) for the denominator
                nc.vector.memset(vnat[:], 1.0)
                for jt in range(JT_QK):
                    for nb in range(2):
                        ps = mmp.tile([128, 512], F32, tag="mm")
                        for kc in range(KC):
                            nc.tensor.matmul(
                                ps[:],
                                wqkv_sb[:, kc, jt * 128:(jt + 1) * 128],
                                xt_sb[:, kc, nb * 512:(nb + 1) * 512],
                                start=(kc == 0), stop=(kc == KC - 1),
                            )
                        nc.vector.tensor_scalar_add(
                            qkT[:, jt, nb * 512:(nb + 1) * 512], ps[:],
                            qkb_sb[:, jt:jt + 1])
                for nt in range(NT):
                    for c0, cw in ((0, 512), (512, 256)):
                        ps = mmp.tile([128, 512], F32, tag="mm")
                        for kc in range(KC):
                            nc.tensor.matmul(
                                ps[:, 0:cw],
                                xt_sb[:, kc, nt * 128:(nt + 1) * 128],
                                wqkv_sb[:, kc, 2 * DIM + c0:2 * DIM + c0 + cw],
                                start=(kc == 0), stop=(kc == KC - 1),
                            )
                        nc.vector.tensor_copy(
                            vnat[:, nt, c0 // HD:(c0 + cw) // HD, 0:HD],
                            ps[:, 0:cw].rearrange("p (h d) -> p h d", d=HD),
                        )
                s_["qkT"] = qkT
                s_["vnat"] = vnat

            # den staging: DVE writes must start at a partition multiple of
            # 32, so head h's denominator goes to partition 32*(h//3), free
            # block h%3; then per-head DMAs repack into den_lo/den_hi rows
            # 0..5 (custom-DVE reciprocal only works at partition base 0).
            def norm_half(b, hlo):
                s_ = st[b]
                den_all = s_["den_lo" if hlo == 0 else "den_hi"]
                recip = s_["recip_lo" if hlo == 0 else "recip_hi"]
                outcat = s_["outcat"]
                nc.vector.reciprocal_approx_accurate(
                    recip[:], den_all[:], s_["den_st"][0:6, 0:N])
                for h in range(hlo, hlo + 6):
                    rb = rbp.tile([128, N], F32, tag="rb")
                    rr = h % 6
                    nc.sync.dma_start(out=rb[0:1, :], in_=recip[rr:rr + 1, :])
                    nc.gpsimd.partition_broadcast(rb[:], rb[0:1, :])
                    p0 = (h % 2) * 64
                    oc_ap = outcat[p0:p0 + 64, h // 2, :]
                    nc.vector.tensor_tensor(oc_ap, oc_ap, rb[p0:p0 + 64, :],
                                            MUL)

            def attn(b):
                s_ = st[b]
                qkT, vnat = s_["qkT"], s_["vnat"]
                outcat = ocp.tile([128, KC, N], DT_AV, tag="outcat")
                den_st = denp.tile([97, 3 * N], F32, tag="denst")
                den_lo = denp.tile([6, N], F32, tag="denlo")
                den_hi = denp.tile([6, N], F32, tag="denhi")
                recip_lo = denp.tile([6, N], F32, tag="reciplo")
                recip_hi = denp.tile([6, N], F32, tag="reciphi")
                s_.update(outcat=outcat, den_st=den_st, den_lo=den_lo,
                          den_hi=den_hi, recip_lo=recip_lo, recip_hi=recip_hi)
                for p in range(6):
                    hA, hB = 2 * p, 2 * p + 1
                    for s in range(2):
                        avA = avp.tile([HD + 1, 512], F32, tag="av")
                        avB = avp.tile([HD + 1, 512], F32, tag="av")
                        for kc in range(8):
                            sc = scp.tile([128, 2, 512], F32, tag="sc")
                            # the two heads' score matmuls run concurrently in
                            # the upper/lower 64 rows of the PE array
                            nc.tensor.matmul(
                                sc[:, 0, :],
                                qkT[0:64, 6 + p, kc * 128:(kc + 1) * 128],
                                qkT[0:64, p, s * 512:(s + 1) * 512],
                                start=True, stop=True)
                            nc.tensor.matmul(
                                sc[:, 1, :],
                                qkT[64:128, 6 + p, kc * 128:(kc + 1) * 128],
                                qkT[64:128, p, s * 512:(s + 1) * 512],
                                start=True, stop=True)
                            et = etp.tile([128, 2, 512], DT_AV, tag="et")
                            nc.scalar.activation(et[:], sc[:], EXP, scale=SCALE)
                            nc.tensor.matmul(
                                avA[:], vnat[:, kc, hA, 0:HD + 1], et[:, 0, :],
                                start=(kc == 0), stop=(kc == 7))
                            nc.tensor.matmul(
                                avB[:], vnat[:, kc, hB, 0:HD + 1], et[:, 1, :],
                                start=(kc == 0), stop=(kc == 7))
                        for h, avt in ((hA, avA), (hB, avB)):
                            p0 = (h % 2) * 64
                            dp = 32 * (h // 3)
                            dc = (h % 3) * N + s * 512
                            nc.vector.tensor_copy(
                                den_st[dp:dp + 1, dc:dc + 512],
                                avt[HD:HD + 1, :])
                            nc.vector.tensor_copy(
                                outcat[p0:p0 + 64, p, s * 512:(s + 1) * 512],
                                avt[0:HD, :])
                    for h in (hA, hB):
                        dtile = den_lo if h < 6 else den_hi
                        rr = h % 6
                        dp = 32 * (h // 3)
                        dc = (h % 3) * N
                        nc.sync.dma_start(out=dtile[rr:rr + 1, :],
                                          in_=den_st[dp:dp + 1, dc:dc + N])
                    if p == 2:
                        norm_half(b, 0)

            def proj(b):
                s_ = st[b]
                outcat = s_["outcat"]
                for nt in range(NT):
                    y_sb = yp.tile([128, DIM], F32, tag="y")
                    for c0, cw in ((0, 512), (512, 256)):
                        ps = mmp.tile([128, 512], F32, tag="mm")
                        for dc in range(KC):
                            nc.tensor.matmul(
                                ps[:, 0:cw],
                                outcat[:, dc, nt * 128:(nt + 1) * 128],
                                wproj_sb[:, dc, c0:c0 + cw],
                                start=(dc == 0), stop=(dc == KC - 1),
                            )
                        nc.vector.tensor_tensor(y_sb[:, c0:c0 + cw],
                                                ps[:, 0:cw],
                                                bias_bc[:, c0:c0 + cw], ADD)
                    nc.sync.dma_start(out=y_d[b, nt * 128:(nt + 1) * 128, :],
                                      in_=y_sb[:])

            # phase order chosen so batch 1's qkv fills batch 0's
            # normalize/proj pipeline gaps
            load(0, with_weights=True)
            qkv(0)
            attn(0)
            load(1)
            qkv(1)
            norm_half(0, 6)
            proj(0)
            attn(1)
            norm_half(1, 6)
            proj(1)

    nc.compile()
    return nc


def _get_nc():
    key = (DT_QK_NAME, DT_AV_NAME)
    if key not in _BUILT:
        _BUILT[key] = _build()
    return _BUILT[key]


def _prep_inputs(x, qkv_w, qkv_b, proj_w, proj_b):
    x = np.asarray(x, dtype=np.float32)
    qkv_w = np.asarray(qkv_w, dtype=np.float32)
    qkv_b = np.asarray(qkv_b, dtype=np.float32)
    proj_w = np.asarray(proj_w, dtype=np.float32)
    proj_b = np.asarray(proj_b, dtype=np.float32)

    wqkvT = _np_cast(np.ascontiguousarray(qkv_w.T), DT_QK_NAME)
    wprojT = _np_cast(np.ascontiguousarray(proj_w.T), DT_AV_NAME)
    qkb = np.ascontiguousarray(qkv_b[:1536].reshape(JT_QK, 128).T)
    bproj = (proj_b + qkv_b[2 * DIM:] @ proj_w.T).reshape(1, DIM)
    bproj = np.ascontiguousarray(bproj, dtype=np.float32)

    in_maps = []
    for c in range(N_CORES):
        xs = x[c * B_LOC:(c + 1) * B_LOC]  # [2, 1024, 768]
        xt = _np_cast(np.ascontiguousarray(xs.transpose(0, 2, 1)), DT_QK_NAME)
        in_maps.append({
            "xt": xt,
            "wqkvT": wqkvT,
            "wprojT": wprojT,
            "qkb": qkb,
            "bproj": bproj,
        })
    return in_maps


def run(x, qkv_w, qkv_b, proj_w, proj_b, **spmd_kwargs):
    """Execute on 8 cores; returns (output, BassKernelResults)."""
    from concourse.bass_utils import run_bass_kernel_spmd

    nc = _get_nc()
    in_maps = _prep_inputs(x, qkv_w, qkv_b, proj_w, proj_b)
    res = run_bass_kernel_spmd(nc, in_maps, core_ids=list(range(N_CORES)),
                               **spmd_kwargs)
    y = np.concatenate([res.results[c]["y"] for c in range(N_CORES)], axis=0)
    return y.astype(np.float32), res


def kernel(x, qkv_w, qkv_b, proj_w, proj_b):
    y, _ = run(x, qkv_w, qkv_b, proj_w, proj_b)
    return y


# revision 15
# speedup vs baseline: 1.5620x; 1.0378x over previous
"""Multi-head attention (B=16, N=1024, dim=768, H=12) on 8 TRN2 NeuronCores.

Sharding: pure data-parallel over batch (2 batches per core). Each core runs
the full attention block on its batch shard; no collectives.

Per-core dataflow (all layouts chosen so no on-device transposes are needed):
  - host pre-transposes x -> xT [768, 1024] per batch and qkv_w/proj_w -> w.T
  - QK projection computed in "T layout": qkT [j, n] (j = head-major rows)
  - V projection computed in natural layout v_nat [n, j] (x used as stationary
    operand), padded to 65 cols per head with a ones column so the attn@v
    matmul also produces the softmax denominator for free
  - scores computed transposed: scT[k, q] = kT.T @ qT, softmax-exp on ACT with
    the 1/sqrt(hd) scale fused (no max subtraction: |scores| <~ 8 for this
    data distribution, exp stays well inside fp32/bf16 range)
  - attn@v: out.T[hd+1, q] = v_nat.T @ expT, row 64 = denominator
  - batched reciprocal (custom DVE op), gpsimd partition-broadcast, in-place
    normalize
  - proj: y[n, dout] = outcatT.T @ projT; V-bias and proj bias folded into a
    single precomputed bias vector added on the way out of PSUM
Precision: f32r (s1e8m11) for the qkv-projection + scores path, bf16 for the
attention-weight/value/proj path (measured end-to-end ~3e-3 relative absmax
vs the fp32 reference, vs ~7e-3 for all-bf16).
"""

import sys

if "/opt/trn_rl_repo" not in sys.path:
    sys.path.insert(0, "/opt/trn_rl_repo")

import numpy as np
import ml_dtypes

N_CORES = 8
B, N, DIM = 16, 1024, 768
H, HD = 12, 64
J = 3 * DIM
SCALE = HD**-0.5
B_LOC = B // N_CORES  # 2 batches per core
NT = N // 128  # 8 n-tiles per batch
KC = DIM // 128  # 6 contraction chunks
JT_QK = 12  # q,k j-tiles (rows 0..1535 of qkv out)

# dtype config: "f32r" or "bf16" for the two halves of the pipeline
DT_QK_NAME = "f32r"  # x, wqkv, q/k activations (scores path)
DT_AV_NAME = "bf16"  # exp weights, v, outcat, wproj (attn-value path)

_BUILT = {}


def _round_f32r(a):
    """Round-to-nearest-even fp32 -> s1e8m11 (what the PE does for float32r)."""
    b = np.ascontiguousarray(a.astype(np.float32)).view(np.uint32)
    low = b & np.uint32(0xFFF)
    hi = b & np.uint32(0xFFFFF000)
    round_up = (low > 0x800) | ((low == 0x800) & (((hi >> 12) & 1) == 1))
    hi = hi + (round_up.astype(np.uint32) << 12)
    return hi.view(np.float32)


def _np_cast(a, name):
    if name == "f32r":
        return _round_f32r(a)
    if name == "bf16":
        return a.astype(ml_dtypes.bfloat16)
    return a.astype(np.float32)


def _build():
    import concourse.bacc as bacc
    import concourse.mybir as mybir
    import concourse.tile as tile

    F32 = mybir.dt.float32
    DT_QK = {"f32r": mybir.dt.float32r, "bf16": mybir.dt.bfloat16}[DT_QK_NAME]
    DT_AV = {"f32r": mybir.dt.float32r, "bf16": mybir.dt.bfloat16}[DT_AV_NAME]
    EXP = mybir.ActivationFunctionType.Exp
    MUL = mybir.AluOpType.mult
    ADD = mybir.AluOpType.add

    nc = bacc.Bacc("TRN2", target_bir_lowering=False, debug=False,
                   num_devices=N_CORES)

    xt_d = nc.dram_tensor("xt", [B_LOC, DIM, N], DT_QK, kind="ExternalInput")
    wqkv_d = nc.dram_tensor("wqkvT", [DIM, J], DT_QK, kind="ExternalInput")
    wproj_d = nc.dram_tensor("wprojT", [DIM, DIM], DT_AV, kind="ExternalInput")
    qkb_d = nc.dram_tensor("qkb", [128, JT_QK], F32, kind="ExternalInput")
    bproj_d = nc.dram_tensor("bproj", [1, DIM], F32, kind="ExternalInput")
    y_d = nc.dram_tensor("y", [B_LOC, N, DIM], F32, kind="ExternalOutput")

    with tile.TileContext(nc) as tc:
        with (
            tc.tile_pool(name="wpool", bufs=1) as wpool,
            tc.tile_pool(name="xtp", bufs=1) as xtp,
            tc.tile_pool(name="qkp", bufs=1) as qkp,
            tc.tile_pool(name="vp", bufs=1) as vp,
            tc.tile_pool(name="ocp", bufs=2) as ocp,
            tc.tile_pool(name="etp", bufs=2) as etp,
            tc.tile_pool(name="denp", bufs=1) as denp,
            tc.tile_pool(name="rbp", bufs=1) as rbp,
            tc.tile_pool(name="yp", bufs=1) as yp,
            tc.tile_pool(name="mmp", bufs=2, space="PSUM") as mmp,
            tc.tile_pool(name="scp", bufs=2, space="PSUM") as scp,
            tc.tile_pool(name="avp", bufs=2, space="PSUM") as avp,
        ):
            wqkv_sb = wpool.tile([128, KC, J], DT_QK)
            wproj_sb = wpool.tile([128, KC, DIM], DT_AV)
            qkb_sb = wpool.tile([128, JT_QK], F32)
            bias_bc = wpool.tile([128, DIM], F32)

            nc.sync.dma_start(out=qkb_sb[:], in_=qkb_d[:])
            nc.sync.dma_start(out=bias_bc[0:1, :], in_=bproj_d[:])
            nc.gpsimd.partition_broadcast(bias_bc[:], bias_bc[0:1, :])

            st = {}  # per-batch tiles

            def load(b, with_weights=False):
                xt_sb = xtp.tile([128, KC, N], DT_QK, tag="xt")
                for kc in range(KC):
                    if with_weights:
                        nc.sync.dma_start(out=wqkv_sb[:, kc, :],
                                          in_=wqkv_d[kc * 128:(kc + 1) * 128, :])
                    nc.sync.dma_start(out=xt_sb[:, kc, :],
                                      in_=xt_d[b, kc * 128:(kc + 1) * 128, :])
                if with_weights:
                    for kc in range(KC):
                        nc.sync.dma_start(out=wproj_sb[:, kc, :],
                                          in_=wproj_d[kc * 128:(kc + 1) * 128, :])
                st[b] = {"xt": xt_sb}

            def qkv(b):
                s_ = st[b]
                xt_sb = s_["xt"]
                qkT = qkp.tile([128, JT_QK, N], DT_QK, tag="qkT")
                vnat = vp.tile([128, NT, H, HD + 1], DT_AV, tag="vnat")
                # ones column (col 64 of every head slot) for the denominator
                nc.vector.memset(vnat[:], 1.0)
                for jt in range(JT_QK):
                    for nb in range(2):
                        ps = mmp.tile([128, 512], F32, tag="mm")
                        for kc in range(KC):
                            nc.tensor.matmul(
                                ps[:],
                                wqkv_sb[:, kc, jt * 128:(jt + 1) * 128],
                                xt_sb[:, kc, nb * 512:(nb + 1) * 512],
                                start=(kc == 0), stop=(kc == KC - 1),
                            )
                        nc.vector.tensor_scalar_add(
                            qkT[:, jt, nb * 512:(nb + 1) * 512], ps[:],
                            qkb_sb[:, jt:jt + 1])
                for nt in range(NT):
                    for c0, cw in ((0, 512), (512, 256)):
                        ps = mmp.tile([128, 512], F32, tag="mm")
                        for kc in range(KC):
                            nc.tensor.matmul(
                                ps[:, 0:cw],
                                xt_sb[:, kc, nt * 128:(nt + 1) * 128],
                                wqkv_sb[:, kc, 2 * DIM + c0:2 * DIM + c0 + cw],
                                start=(kc == 0), stop=(kc == KC - 1),
                            )
                        nc.vector.tensor_copy(
                            vnat[:, nt, c0 // HD:(c0 + cw) // HD, 0:HD],
                            ps[:, 0:cw].rearrange("p (h d) -> p h d", d=HD),
                        )
                s_["qkT"] = qkT
                s_["vnat"] = vnat

            # den staging: DVE writes must start at a partition multiple of
            # 32, so head h's denominator goes to partition 32*(h//3), free
            # block h%3; then per-head DMAs repack into den_lo/den_hi rows
            # 0..5 (custom-DVE reciprocal only works at partition base 0).
            def norm_half(b, hlo):
                s_ = st[b]
                # reciprocal computed in place over the staged denominators
                recip = s_["den_lo" if hlo == 0 else "den_hi"]
                outcat = s_["outcat"]
                nc.vector.reciprocal_approx_accurate(
                    recip[:], recip[:], s_["den_st"][0:6, 0:N])
                for h in range(hlo, hlo + 6):
                    rb = rbp.tile([128, N], F32, tag="rb")
                    rr = h % 6
                    nc.sync.dma_start(out=rb[0:1, :], in_=recip[rr:rr + 1, :])
                    nc.gpsimd.partition_broadcast(rb[:], rb[0:1, :])
                    p0 = (h % 2) * 64
                    oc_ap = outcat[p0:p0 + 64, h // 2, :]
                    nc.vector.tensor_tensor(oc_ap, oc_ap, rb[p0:p0 + 64, :],
                                            MUL)

            def attn(b):
                s_ = st[b]
                qkT, vnat = s_["qkT"], s_["vnat"]
                outcat = ocp.tile([128, KC, N], DT_AV, tag="outcat")
                den_st = denp.tile([97, 3 * N], F32, tag="denst")
                den_lo = denp.tile([6, N], F32, tag="denlo")
                den_hi = denp.tile([6, N], F32, tag="denhi")
                s_.update(outcat=outcat, den_st=den_st, den_lo=den_lo,
                          den_hi=den_hi)
                for p in range(6):
                    hA, hB = 2 * p, 2 * p + 1
                    for s in range(2):
                        avA = avp.tile([HD + 1, 512], F32, tag="av")
                        avB = avp.tile([HD + 1, 512], F32, tag="av")
                        for kc in range(8):
                            sc = scp.tile([128, 2, 512], F32, tag="sc")
                            # the two heads' score matmuls run concurrently in
                            # the upper/lower 64 rows of the PE array
                            nc.tensor.matmul(
                                sc[:, 0, :],
                                qkT[0:64, 6 + p, kc * 128:(kc + 1) * 128],
                                qkT[0:64, p, s * 512:(s + 1) * 512],
                                start=True, stop=True)
                            nc.tensor.matmul(
                                sc[:, 1, :],
                                qkT[64:128, 6 + p, kc * 128:(kc + 1) * 128],
                                qkT[64:128, p, s * 512:(s + 1) * 512],
                                start=True, stop=True)
                            et = etp.tile([128, 2, 512], DT_AV, tag="et")
                            nc.scalar.activation(et[:], sc[:], EXP, scale=SCALE)
                            nc.tensor.matmul(
                                avA[:], vnat[:, kc, hA, 0:HD + 1], et[:, 0, :],
                                start=(kc == 0), stop=(kc == 7))
                            nc.tensor.matmul(
                                avB[:], vnat[:, kc, hB, 0:HD + 1], et[:, 1, :],
                                start=(kc == 0), stop=(kc == 7))
                        for h, avt in ((hA, avA), (hB, avB)):
                            p0 = (h % 2) * 64
                            dp = 32 * (h // 3)
                            dc = (h % 3) * N + s * 512
                            nc.vector.tensor_copy(
                                den_st[dp:dp + 1, dc:dc + 512],
                                avt[HD:HD + 1, :])
                            nc.vector.tensor_copy(
                                outcat[p0:p0 + 64, p, s * 512:(s + 1) * 512],
                                avt[0:HD, :])
                    for h in (hA, hB):
                        dtile = den_lo if h < 6 else den_hi
                        rr = h % 6
                        dp = 32 * (h // 3)
                        dc = (h % 3) * N
                        nc.sync.dma_start(out=dtile[rr:rr + 1, :],
                                          in_=den_st[dp:dp + 1, dc:dc + N])
                    if p == 2:
                        norm_half(b, 0)

            def proj(b):
                s_ = st[b]
                outcat = s_["outcat"]
                for nt in range(NT):
                    y_sb = yp.tile([128, DIM], F32, tag="y")
                    for c0, cw in ((0, 512), (512, 256)):
                        ps = mmp.tile([128, 512], F32, tag="mm")
                        for dc in range(KC):
                            nc.tensor.matmul(
                                ps[:, 0:cw],
                                outcat[:, dc, nt * 128:(nt + 1) * 128],
                                wproj_sb[:, dc, c0:c0 + cw],
                                start=(dc == 0), stop=(dc == KC - 1),
                            )
                        nc.vector.tensor_tensor(y_sb[:, c0:c0 + cw],
                                                ps[:, 0:cw],
                                                bias_bc[:, c0:c0 + cw], ADD)
                    nc.sync.dma_start(out=y_d[b, nt * 128:(nt + 1) * 128, :],
                                      in_=y_sb[:])

            # phase order chosen so batch 1's qkv fills batch 0's
            # normalize/proj pipeline gaps
            load(0, with_weights=True)
            qkv(0)
            attn(0)
            load(1)
            qkv(1)
            norm_half(0, 6)
            attn(1)
            proj(0)
            norm_half(1, 6)
            proj(1)

    nc.compile()
    return nc


def _get_nc():
    key = (DT_QK_NAME, DT_AV_NAME)
    if key not in _BUILT:
        _BUILT[key] = _build()
    return _BUILT[key]


def _prep_inputs(x, qkv_w, qkv_b, proj_w, proj_b):
    x = np.asarray(x, dtype=np.float32)
    qkv_w = np.asarray(qkv_w, dtype=np.float32)
    qkv_b = np.asarray(qkv_b, dtype=np.float32)
    proj_w = np.asarray(proj_w, dtype=np.float32)
    proj_b = np.asarray(proj_b, dtype=np.float32)

    wqkvT = _np_cast(np.ascontiguousarray(qkv_w.T), DT_QK_NAME)
    wprojT = _np_cast(np.ascontiguousarray(proj_w.T), DT_AV_NAME)
    qkb = np.ascontiguousarray(qkv_b[:1536].reshape(JT_QK, 128).T)
    bproj = (proj_b + qkv_b[2 * DIM:] @ proj_w.T).reshape(1, DIM)
    bproj = np.ascontiguousarray(bproj, dtype=np.float32)

    in_maps = []
    for c in range(N_CORES):
        xs = x[c * B_LOC:(c + 1) * B_LOC]  # [2, 1024, 768]
        xt = _np_cast(np.ascontiguousarray(xs.transpose(0, 2, 1)), DT_QK_NAME)
        in_maps.append({
            "xt": xt,
            "wqkvT": wqkvT,
            "wprojT": wprojT,
            "qkb": qkb,
            "bproj": bproj,
        })
    return in_maps


def run(x, qkv_w, qkv_b, proj_w, proj_b, **spmd_kwargs):
    """Execute on 8 cores; returns (output, BassKernelResults)."""
    from concourse.bass_utils import run_bass_kernel_spmd

    nc = _get_nc()
    in_maps = _prep_inputs(x, qkv_w, qkv_b, proj_w, proj_b)
    res = run_bass_kernel_spmd(nc, in_maps, core_ids=list(range(N_CORES)),
                               **spmd_kwargs)
    y = np.concatenate([res.results[c]["y"] for c in range(N_CORES)], axis=0)
    return y.astype(np.float32), res


def kernel(x, qkv_w, qkv_b, proj_w, proj_b):
    y, _ = run(x, qkv_w, qkv_b, proj_w, proj_b)
    return y
